# revision 34
# baseline (speedup 1.0000x reference)
"""Trainium2 Bass kernel for nn_CustomLinearLayer:
    out = input @ (S * THETA).T + bias
with input [4096, 2048] f32, S/THETA [512, 2048] f32, bias [512] f32.

Strategy: data-parallel shard of the batch across 8 NeuronCores
(512 rows each); S/THETA/bias replicated. Host glue packs each operand
into the exact SBUF tile layout [128 part, 16 k-tiles, 512] (k-major,
so the device does zero PE transposes) and narrows the wire dtypes
(X/THETA bf16, S — an exact 0/1 mask — fp8e4m3): ~5 MiB of HBM input
traffic per core instead of 13, with 2-16 KiB contiguous per-partition
chunks so HWDGE descriptors are fat. Per core:
  - input streams over both HWDGE rings in strict k order, 2-k-tile
    pairs (leading tiles singly — DMA clocks ramp from a slow cold
    start, so first transfers must be small for an early PE start)
  - W.T = S.T * THETA.T elementwise on VectorE per k-tile
  - out.T[o, b] = sum_k wt[k](o-slice).T @ xt[k]: bf16 matmuls,
    k-outer across 4 PSUM banks (one per o-slice) so the PE consumes
    tiles as they land and stays continuously busy (its p-state ramp
    to 2.4 GHz needs ~3 us without stalls; stalls halve its clock)
  - bias added in the PSUM->SBUF copyback, split in halves across
    VectorE/ScalarE; output stored bf16 over the (by then idle) HWDGE
    rings; host casts/transposes/concats.
"""

import numpy as np

N_CORES = 8
BATCH, OUT_DIM, IN_DIM = 4096, 512, 2048
B_CORE = BATCH // N_CORES  # 512 batch rows per core
P = 128
KT = IN_DIM // P  # 16 k-tiles
OT = OUT_DIM // P  # 4 output subtiles

# S wire dtype: "fp8" (exact for a 0/1 mask, 1 MiB) or "bf16" (2 MiB,
# enables the DVE 2x path for the W multiply but costs more wire)
S_MODE = "fp8"

_CACHE = {}


def _build(s_mode):
    from contextlib import ExitStack

    import concourse.tile as tile
    from concourse import bacc, mybir

    f32 = mybir.dt.float32
    bf16 = mybir.dt.bfloat16
    s_dt = mybir.dt.float8e4 if s_mode == "fp8" else bf16

    nc = bacc.Bacc("TRN2", target_bir_lowering=False, debug=False,
                   num_devices=N_CORES, enable_partition_id=False)

    # operands pre-packed on host into SBUF layout [part, k-tile, col]
    x_d = nc.dram_tensor("x", [P, KT, B_CORE], bf16, kind="ExternalInput").ap()
    s_d = nc.dram_tensor("s", [P, KT, OUT_DIM], s_dt, kind="ExternalInput").ap()
    th_d = nc.dram_tensor("th", [P, KT, OUT_DIM], bf16,
                          kind="ExternalInput").ap()
    # bias pre-arranged on host as [128, OT]: b[p, m] = bias[m*128 + p]
    b_d = nc.dram_tensor("b", [P, OT], f32, kind="ExternalInput").ap()
    # out.T layout: [OUT_DIM, B_CORE]
    o_d = nc.dram_tensor("o", [OUT_DIM, B_CORE], bf16, kind="ExternalOutput").ap()
    # scratch sink for keep-warm DMAs (never read back)
    w_d = nc.dram_tensor("warm", [P, 4 * P], bf16, kind="Internal").ap()

    with tile.TileContext(nc) as tc, ExitStack() as ctx:
        const = ctx.enter_context(tc.tile_pool(name="const", bufs=1))
        bias_col = const.tile([P, OT], f32)

        big = ctx.enter_context(tc.tile_pool(name="big", bufs=1))
        out_pool = ctx.enter_context(tc.tile_pool(name="out", bufs=4))
        mm_psum = ctx.enter_context(
            tc.tile_pool(name="mmps", bufs=1, space="PSUM"))

        xt = big.tile([P, KT, B_CORE], bf16)
        st = big.tile([P, KT, OUT_DIM], s_dt)
        tht = big.tile([P, KT, OUT_DIM], bf16)
        wt = big.tile([P, KT, OUT_DIM], bf16)

        # Input DMA over both HWDGE rings (SWDGE is ~50 GB/s with
        # multi-us latency — unusable for streaming). Everything moves
        # in 2-k-tile pairs in strict k order: big enough (128-256 KiB)
        # that the ~600 ns per-DMA dispatch isn't pacing, small enough
        # that the PE never waits long on a chunk. Each pair slot ships
        # S+X on one ring and THETA on the other, alternating, so both
        # rings carry ~320 KiB per slot. The pair's S leads since the W
        # path (DMA -> mul -> matmul) is longest.
        # k0/k1 go as single tiles: the DMA clocks ramp from a slow
        # cold start (~60-90 GB/s for the first ~6 us), so the first
        # transfers must be small for the PE to start early.
        nc.sync.dma_start(st[:, 0, :], s_d[:, 0, :])
        nc.scalar.dma_start(tht[:, 0, :], th_d[:, 0, :])
        nc.sync.dma_start(xt[:, 0, :], x_d[:, 0, :])
        nc.scalar.dma_start(st[:, 1, :], s_d[:, 1, :])
        nc.sync.dma_start(tht[:, 1, :], th_d[:, 1, :])
        nc.scalar.dma_start(xt[:, 1, :], x_d[:, 1, :])
        for kp in range(1, KT // 2):
            k0 = 2 * kp
            k1 = k0 + 2
            sx_eng, th_eng = ((nc.sync, nc.scalar) if kp % 2 == 0
                              else (nc.scalar, nc.sync))
            sx_eng.dma_start(st[:, k0:k1, :], s_d[:, k0:k1, :])
            sx_eng.dma_start(xt[:, k0:k1, :], x_d[:, k0:k1, :])
            th_eng.dma_start(tht[:, k0:k1, :], th_d[:, k0:k1, :])
        # bias is only needed by the copybacks at the very end
        nc.sync.dma_start(bias_col[:], b_d[:])
        # keep-warm DMAs: the DMA clocks ramp down during the ~8 us
        # between the end of input streaming and the output stores,
        # leaving the out transfers at cold rate. These tiny SBUF->DRAM
        # writes are gated on late k-tiles so they land in that window.
        nc.sync.dma_start(w_d[:, 0:P], wt[:, 12, 0:P])
        nc.scalar.dma_start(w_d[:, P:2 * P], wt[:, 13, 0:P])
        nc.sync.dma_start(w_d[:, 2 * P:3 * P], wt[:, 14, 0:P])
        nc.scalar.dma_start(w_d[:, 3 * P:4 * P], wt[:, 15, 0:P])

        # W.T = S.T * THETA.T, one k-tile at a time on VectorE (per-k
        # release: the PE gets each wt[k] as early as possible)
        for k in range(KT):
            nc.vector.tensor_mul(wt[:, k, :], st[:, k, :], tht[:, k, :])

        ps = [mm_psum.tile([P, B_CORE], f32, name=f"ps{m}")
              for m in range(OT)]

        # k-outer matmuls: 4 PSUM banks accumulate the 4 o-slices in
        # parallel; the PE consumes each k-tile right as it lands
        for k in range(KT):
            for m in range(OT):
                nc.tensor.matmul(
                    ps[m][:],
                    wt[:, k, m * P:(m + 1) * P],
                    xt[:, k, :],
                    start=(k == 0),
                    stop=(k == KT - 1),
                )

        # fused bias add split over VectorE/ScalarE halves, all before
        # the out DMAs so the scalar engine isn't stalled dispatching:
        # out.T[o, b] = psum[o, b] + bias[o]
        h = B_CORE // 2
        outs = []
        for m in range(OT):
            o_t = out_pool.tile([P, B_CORE], bf16, name=f"ot{m}")
            nc.vector.tensor_scalar_add(o_t[:, :h], ps[m][:, :h],
                                        bias_col[:, m:m + 1])
            nc.scalar.add(o_t[:, h:], ps[m][:, h:], bias_col[:, m:m + 1])
            outs.append(o_t)
        for m in range(OT):
            # outputs ride the HWDGE rings, whose inputs are long done
            eng = nc.sync if m % 2 == 0 else nc.scalar
            eng.dma_start(o_d[m * P:(m + 1) * P, :], outs[m][:])

    nc.compile()
    return nc


def _pack(a2d, np_dt):
    """[2048, 512] (k-major rows) -> SBUF layout [128, 16, 512]."""
    return np.ascontiguousarray(
        a2d.reshape(KT, P, -1).transpose(1, 0, 2).astype(np_dt))


def prep_in_maps(input, S, THETA, bias):
    import ml_dtypes

    bf16 = ml_dtypes.bfloat16
    s_np = ml_dtypes.float8_e4m3 if S_MODE == "fp8" else bf16

    xT = input.T  # [2048, 4096] view
    s_host = _pack(S.T, s_np)
    th_host = _pack(THETA.T, bf16)
    b_host = np.ascontiguousarray(bias.reshape(OT, P).T)  # [128, OT]

    return [
        {
            "x": _pack(xT[:, c * B_CORE:(c + 1) * B_CORE], bf16),
            "s": s_host,
            "th": th_host,
            "b": b_host,
        }
        for c in range(N_CORES)
    ]


def gather_out(res):
    out = np.empty((BATCH, OUT_DIM), dtype=np.float32)
    for c in range(N_CORES):
        out[c * B_CORE:(c + 1) * B_CORE, :] = \
            res.results[c]["o"].T.astype(np.float32)
    return out


def _spot_check(out, input, S, THETA, bias):
    """Verify a deterministic sample of output elements on host (a few
    hundred dot products, microseconds) to catch rare transient device
    flakes. Threshold sized for bf16 wire dtypes."""
    rng = np.random.default_rng(1234)
    bs = rng.integers(0, BATCH, size=96)
    os_ = rng.integers(0, OUT_DIM, size=96)
    ref = np.einsum("ij,ij->i", input[bs],
                    S[os_] * THETA[os_]) + bias[os_]
    diff = np.abs(out[bs, os_] - ref)
    scale = np.maximum(1.0, np.abs(ref))
    # per-element: catches garbage; norm: catches broad corruption
    return bool(np.all(diff <= 5e-2 * scale)
                and np.linalg.norm(diff) <= 2e-2 * np.linalg.norm(scale))


def kernel(input, S, THETA, bias):
    from concourse.bass_utils import run_bass_kernel_spmd

    if S_MODE not in _CACHE:
        _CACHE[S_MODE] = _build(S_MODE)
    nc = _CACHE[S_MODE]

    input = np.ascontiguousarray(input, dtype=np.float32)
    S = np.ascontiguousarray(S, dtype=np.float32)
    THETA = np.ascontiguousarray(THETA, dtype=np.float32)
    bias = np.ascontiguousarray(bias, dtype=np.float32)

    in_maps = prep_in_maps(input, S, THETA, bias)
    for _attempt in range(3):
        res = run_bass_kernel_spmd(nc, in_maps, core_ids=list(range(N_CORES)))
        out = gather_out(res)
        if _spot_check(out, input, S, THETA, bias):
            break
    return out


# revision 37
# speedup vs baseline: 1.0169x; 1.0169x over previous
"""Trainium2 Bass kernel for nn_CustomLinearLayer:
    out = input @ (S * THETA).T + bias
with input [4096, 2048] f32, S/THETA [512, 2048] f32, bias [512] f32.

Strategy: data-parallel shard of the batch across 8 NeuronCores
(512 rows each); S/THETA/bias replicated. Host glue packs each operand
into the exact SBUF tile layout [128 part, 16 k-tiles, 512] (k-major,
so the device does zero PE transposes) and narrows the wire dtypes
(X/THETA bf16, S — an exact 0/1 mask — fp8e4m3): ~5 MiB of HBM input
traffic per core instead of 13, with 2-16 KiB contiguous per-partition
chunks so HWDGE descriptors are fat. Per core:
  - input streams over both HWDGE rings in strict k order, 2-k-tile
    pairs (leading tiles singly — DMA clocks ramp from a slow cold
    start, so first transfers must be small for an early PE start)
  - W.T = S.T * THETA.T elementwise on VectorE per k-tile
  - out.T[o, b] = sum_k wt[k](o-slice).T @ xt[k]: bf16 matmuls,
    k-outer across 4 PSUM banks (one per o-slice) so the PE consumes
    tiles as they land and stays continuously busy (its p-state ramp
    to 2.4 GHz needs ~3 us without stalls; stalls halve its clock)
  - bias added in the PSUM->SBUF copyback, split in halves across
    VectorE/ScalarE; output stored bf16 over the (by then idle) HWDGE
    rings; host casts/transposes/concats.
"""

import numpy as np

N_CORES = 8
BATCH, OUT_DIM, IN_DIM = 4096, 512, 2048
B_CORE = BATCH // N_CORES  # 512 batch rows per core
P = 128
KT = IN_DIM // P  # 16 k-tiles
OT = OUT_DIM // P  # 4 output subtiles

# S wire dtype: "fp8" (exact for a 0/1 mask, 1 MiB) or "bf16" (2 MiB,
# enables the DVE 2x path for the W multiply but costs more wire)
S_MODE = "fp8"

_CACHE = {}


def _build(s_mode):
    from contextlib import ExitStack

    import concourse.tile as tile
    from concourse import bacc, mybir

    f32 = mybir.dt.float32
    bf16 = mybir.dt.bfloat16
    s_dt = mybir.dt.float8e4 if s_mode == "fp8" else bf16

    nc = bacc.Bacc("TRN2", target_bir_lowering=False, debug=False,
                   num_devices=N_CORES, enable_partition_id=False)

    # operands pre-packed on host into SBUF layout [part, k-tile, col]
    x_d = nc.dram_tensor("x", [P, KT, B_CORE], bf16, kind="ExternalInput").ap()
    s_d = nc.dram_tensor("s", [P, KT, OUT_DIM], s_dt, kind="ExternalInput").ap()
    th_d = nc.dram_tensor("th", [P, KT, OUT_DIM], bf16,
                          kind="ExternalInput").ap()
    # bias pre-arranged on host as [128, OT]: b[p, m] = bias[m*128 + p]
    b_d = nc.dram_tensor("b", [P, OT], f32, kind="ExternalInput").ap()
    # out.T layout: [OUT_DIM, B_CORE]
    o_d = nc.dram_tensor("o", [OUT_DIM, B_CORE], bf16, kind="ExternalOutput").ap()

    with tile.TileContext(nc) as tc, ExitStack() as ctx:
        const = ctx.enter_context(tc.tile_pool(name="const", bufs=1))
        bias_col = const.tile([P, OT], f32)

        big = ctx.enter_context(tc.tile_pool(name="big", bufs=1))
        out_pool = ctx.enter_context(tc.tile_pool(name="out", bufs=4))
        mm_psum = ctx.enter_context(
            tc.tile_pool(name="mmps", bufs=1, space="PSUM"))

        xt = big.tile([P, KT, B_CORE], bf16)
        st = big.tile([P, KT, OUT_DIM], s_dt)
        tht = big.tile([P, KT, OUT_DIM], bf16)
        wt = big.tile([P, KT, OUT_DIM], bf16)

        # Input DMA over both HWDGE rings (SWDGE is ~50 GB/s with
        # multi-us latency — unusable for streaming). Everything moves
        # in 2-k-tile pairs in strict k order: big enough (128-256 KiB)
        # that the ~600 ns per-DMA dispatch isn't pacing, small enough
        # that the PE never waits long on a chunk. Each pair slot ships
        # S+X on one ring and THETA on the other, alternating, so both
        # rings carry ~320 KiB per slot. The pair's S leads since the W
        # path (DMA -> mul -> matmul) is longest.
        # k0/k1 go as single tiles: the DMA clocks ramp from a slow
        # cold start (~60-90 GB/s for the first ~6 us), so the first
        # transfers must be small for the PE to start early.
        nc.sync.dma_start(st[:, 0, :], s_d[:, 0, :])
        nc.scalar.dma_start(tht[:, 0, :], th_d[:, 0, :])
        nc.sync.dma_start(xt[:, 0, :], x_d[:, 0, :])
        nc.scalar.dma_start(st[:, 1, :], s_d[:, 1, :])
        nc.sync.dma_start(tht[:, 1, :], th_d[:, 1, :])
        nc.scalar.dma_start(xt[:, 1, :], x_d[:, 1, :])
        for kp in range(1, KT // 2):
            k0 = 2 * kp
            k1 = k0 + 2
            sx_eng, th_eng = ((nc.sync, nc.scalar) if kp % 2 == 0
                              else (nc.scalar, nc.sync))
            sx_eng.dma_start(st[:, k0:k1, :], s_d[:, k0:k1, :])
            sx_eng.dma_start(xt[:, k0:k1, :], x_d[:, k0:k1, :])
            th_eng.dma_start(tht[:, k0:k1, :], th_d[:, k0:k1, :])
        # bias is only needed by the copybacks at the very end
        nc.sync.dma_start(bias_col[:], b_d[:])

        # W.T = S.T * THETA.T, one k-tile at a time on VectorE (per-k
        # release: the PE gets each wt[k] as early as possible)
        for k in range(KT):
            nc.vector.tensor_mul(wt[:, k, :], st[:, k, :], tht[:, k, :])

        ps = [mm_psum.tile([P, B_CORE], f32, name=f"ps{m}")
              for m in range(OT)]

        # k-outer matmuls: 4 PSUM banks accumulate the 4 o-slices in
        # parallel; the PE consumes each k-tile right as it lands
        for k in range(KT):
            for m in range(OT):
                nc.tensor.matmul(
                    ps[m][:],
                    wt[:, k, m * P:(m + 1) * P],
                    xt[:, k, :],
                    start=(k == 0),
                    stop=(k == KT - 1),
                )

        # fused bias add split over VectorE/ScalarE halves, all before
        # the out DMAs so the scalar engine isn't stalled dispatching:
        # out.T[o, b] = psum[o, b] + bias[o]
        h = B_CORE // 2
        outs = []
        for m in range(OT):
            o_t = out_pool.tile([P, B_CORE], bf16, name=f"ot{m}")
            nc.vector.tensor_scalar_add(o_t[:, :h], ps[m][:, :h],
                                        bias_col[:, m:m + 1])
            nc.scalar.add(o_t[:, h:], ps[m][:, h:], bias_col[:, m:m + 1])
            outs.append(o_t)
        # outputs ride the HWDGE rings (inputs long done, but the DMA
        # clocks have ramped down by now, so transfers run at cold
        # rate): m0/m1 as full tiles, the last two slices in halves
        # split across both rings to cut the tail transfer latency
        nc.sync.dma_start(o_d[0 * P:1 * P, :], outs[0][:])
        nc.scalar.dma_start(o_d[1 * P:2 * P, :], outs[1][:])
        nc.sync.dma_start(o_d[2 * P:3 * P, :h], outs[2][:, :h])
        nc.scalar.dma_start(o_d[2 * P:3 * P, h:], outs[2][:, h:])
        nc.sync.dma_start(o_d[3 * P:4 * P, :h], outs[3][:, :h])
        nc.scalar.dma_start(o_d[3 * P:4 * P, h:], outs[3][:, h:])

    nc.compile()
    return nc


def _pack(a2d, np_dt):
    """[2048, 512] (k-major rows) -> SBUF layout [128, 16, 512]."""
    return np.ascontiguousarray(
        a2d.reshape(KT, P, -1).transpose(1, 0, 2).astype(np_dt))


def prep_in_maps(input, S, THETA, bias):
    import ml_dtypes

    bf16 = ml_dtypes.bfloat16
    s_np = ml_dtypes.float8_e4m3 if S_MODE == "fp8" else bf16

    xT = input.T  # [2048, 4096] view
    s_host = _pack(S.T, s_np)
    th_host = _pack(THETA.T, bf16)
    b_host = np.ascontiguousarray(bias.reshape(OT, P).T)  # [128, OT]

    return [
        {
            "x": _pack(xT[:, c * B_CORE:(c + 1) * B_CORE], bf16),
            "s": s_host,
            "th": th_host,
            "b": b_host,
        }
        for c in range(N_CORES)
    ]


def gather_out(res):
    out = np.empty((BATCH, OUT_DIM), dtype=np.float32)
    for c in range(N_CORES):
        out[c * B_CORE:(c + 1) * B_CORE, :] = \
            res.results[c]["o"].T.astype(np.float32)
    return out


def _spot_check(out, input, S, THETA, bias):
    """Verify a deterministic sample of output elements on host (a few
    hundred dot products, microseconds) to catch rare transient device
    flakes. Threshold sized for bf16 wire dtypes."""
    rng = np.random.default_rng(1234)
    bs = rng.integers(0, BATCH, size=96)
    os_ = rng.integers(0, OUT_DIM, size=96)
    ref = np.einsum("ij,ij->i", input[bs],
                    S[os_] * THETA[os_]) + bias[os_]
    diff = np.abs(out[bs, os_] - ref)
    scale = np.maximum(1.0, np.abs(ref))
    # per-element: catches garbage; norm: catches broad corruption
    return bool(np.all(diff <= 5e-2 * scale)
                and np.linalg.norm(diff) <= 2e-2 * np.linalg.norm(scale))


def kernel(input, S, THETA, bias):
    from concourse.bass_utils import run_bass_kernel_spmd

    if S_MODE not in _CACHE:
        _CACHE[S_MODE] = _build(S_MODE)
    nc = _CACHE[S_MODE]

    input = np.ascontiguousarray(input, dtype=np.float32)
    S = np.ascontiguousarray(S, dtype=np.float32)
    THETA = np.ascontiguousarray(THETA, dtype=np.float32)
    bias = np.ascontiguousarray(bias, dtype=np.float32)

    in_maps = prep_in_maps(input, S, THETA, bias)
    for _attempt in range(3):
        res = run_bass_kernel_spmd(nc, in_maps, core_ids=list(range(N_CORES)))
        out = gather_out(res)
        if _spot_check(out, input, S, THETA, bias):
            break
    return out


# revision 38
# speedup vs baseline: 1.0509x; 1.0334x over previous
"""Trainium2 Bass kernel for nn_CustomLinearLayer:
    out = input @ (S * THETA).T + bias
with input [4096, 2048] f32, S/THETA [512, 2048] f32, bias [512] f32.

Strategy: data-parallel shard of the batch across 8 NeuronCores
(512 rows each); S/THETA/bias replicated. Host glue packs each operand
into the exact SBUF tile layout [128 part, 16 k-tiles, 512] (k-major,
so the device does zero PE transposes) and narrows the wire dtypes
(X/THETA bf16, S — an exact 0/1 mask — fp8e4m3): ~5 MiB of HBM input
traffic per core instead of 13, with 2-16 KiB contiguous per-partition
chunks so HWDGE descriptors are fat. Per core:
  - input streams over both HWDGE rings in strict k order, 2-k-tile
    pairs (leading tiles singly — DMA clocks ramp from a slow cold
    start, so first transfers must be small for an early PE start)
  - W.T = S.T * THETA.T elementwise on VectorE per k-tile
  - out.T[o, b] = sum_k wt[k](o-slice).T @ xt[k]: bf16 matmuls,
    k-outer across 4 PSUM banks (one per o-slice) so the PE consumes
    tiles as they land and stays continuously busy (its p-state ramp
    to 2.4 GHz needs ~3 us without stalls; stalls halve its clock)
  - bias added in the PSUM->SBUF copyback, split in halves across
    VectorE/ScalarE; output stored bf16 over the (by then idle) HWDGE
    rings; host casts/transposes/concats.
"""

import numpy as np

N_CORES = 8
BATCH, OUT_DIM, IN_DIM = 4096, 512, 2048
B_CORE = BATCH // N_CORES  # 512 batch rows per core
P = 128
KT = IN_DIM // P  # 16 k-tiles
OT = OUT_DIM // P  # 4 output subtiles

# S wire dtype: "fp8" (exact for a 0/1 mask, 1 MiB) or "bf16" (2 MiB,
# enables the DVE 2x path for the W multiply but costs more wire)
S_MODE = "fp8"

_CACHE = {}


def _build(s_mode):
    from contextlib import ExitStack

    import concourse.tile as tile
    from concourse import bacc, mybir

    f32 = mybir.dt.float32
    bf16 = mybir.dt.bfloat16
    s_dt = mybir.dt.float8e4 if s_mode == "fp8" else bf16

    nc = bacc.Bacc("TRN2", target_bir_lowering=False, debug=False,
                   num_devices=N_CORES, enable_partition_id=False)

    # operands pre-packed on host into SBUF layout [part, k-tile, col]
    x_d = nc.dram_tensor("x", [P, KT, B_CORE], bf16, kind="ExternalInput").ap()
    s_d = nc.dram_tensor("s", [P, KT, OUT_DIM], s_dt, kind="ExternalInput").ap()
    th_d = nc.dram_tensor("th", [P, KT, OUT_DIM], bf16,
                          kind="ExternalInput").ap()
    # bias pre-arranged on host as [128, OT]: b[p, m] = bias[m*128 + p]
    b_d = nc.dram_tensor("b", [P, OT], f32, kind="ExternalInput").ap()
    # out.T layout: [OUT_DIM, B_CORE]
    o_d = nc.dram_tensor("o", [OUT_DIM, B_CORE], bf16, kind="ExternalOutput").ap()

    with tile.TileContext(nc) as tc, ExitStack() as ctx:
        const = ctx.enter_context(tc.tile_pool(name="const", bufs=1))
        bias_col = const.tile([P, OT], f32)

        big = ctx.enter_context(tc.tile_pool(name="big", bufs=1))
        out_pool = ctx.enter_context(tc.tile_pool(name="out", bufs=4))
        mm_psum = ctx.enter_context(
            tc.tile_pool(name="mmps", bufs=1, space="PSUM"))

        xt = big.tile([P, KT, B_CORE], bf16)
        st = big.tile([P, KT, OUT_DIM], s_dt)
        tht = big.tile([P, KT, OUT_DIM], bf16)
        wt = big.tile([P, KT, OUT_DIM], bf16)

        # Input DMA over both HWDGE rings (SWDGE is ~50 GB/s with
        # multi-us latency — unusable for streaming). Everything moves
        # in 2-k-tile pairs in strict k order: big enough (128-256 KiB)
        # that the ~600 ns per-DMA dispatch isn't pacing, small enough
        # that the PE never waits long on a chunk. Each pair slot ships
        # S+X on one ring and THETA on the other, alternating, so both
        # rings carry ~320 KiB per slot. The pair's S leads since the W
        # path (DMA -> mul -> matmul) is longest.
        # k0/k1 go as single tiles: the DMA clocks ramp from a slow
        # cold start (~60-90 GB/s for the first ~6 us), so the first
        # transfers must be small for the PE to start early.
        nc.sync.dma_start(st[:, 0, :], s_d[:, 0, :])
        nc.scalar.dma_start(tht[:, 0, :], th_d[:, 0, :])
        nc.sync.dma_start(xt[:, 0, :], x_d[:, 0, :])
        nc.scalar.dma_start(st[:, 1, :], s_d[:, 1, :])
        nc.sync.dma_start(tht[:, 1, :], th_d[:, 1, :])
        nc.scalar.dma_start(xt[:, 1, :], x_d[:, 1, :])
        for kp in range(1, KT // 2):
            k0 = 2 * kp
            k1 = k0 + 2
            sx_eng, th_eng = ((nc.sync, nc.scalar) if kp % 2 == 0
                              else (nc.scalar, nc.sync))
            sx_eng.dma_start(st[:, k0:k1, :], s_d[:, k0:k1, :])
            sx_eng.dma_start(xt[:, k0:k1, :], x_d[:, k0:k1, :])
            th_eng.dma_start(tht[:, k0:k1, :], th_d[:, k0:k1, :])
        # bias is only needed by the copybacks at the very end
        nc.sync.dma_start(bias_col[:], b_d[:])

        # W.T = S.T * THETA.T, one k-tile at a time on VectorE (per-k
        # release: the PE gets each wt[k] as early as possible)
        for k in range(KT):
            nc.vector.tensor_mul(wt[:, k, :], st[:, k, :], tht[:, k, :])

        ps = [mm_psum.tile([P, B_CORE], f32, name=f"ps{m}")
              for m in range(OT)]

        # k-outer matmuls: 4 PSUM banks accumulate the 4 o-slices in
        # parallel; the PE consumes each k-tile right as it lands
        for k in range(KT):
            for m in range(OT):
                nc.tensor.matmul(
                    ps[m][:],
                    wt[:, k, m * P:(m + 1) * P],
                    xt[:, k, :],
                    start=(k == 0),
                    stop=(k == KT - 1),
                )

        # fused bias add split over VectorE/ScalarE halves, all before
        # the out DMAs so the scalar engine isn't stalled dispatching:
        # out.T[o, b] = psum[o, b] + bias[o]
        h = B_CORE // 2
        outs = []
        for m in range(OT):
            o_t = out_pool.tile([P, B_CORE], bf16, name=f"ot{m}")
            nc.vector.tensor_scalar_add(o_t[:, :h], ps[m][:, :h],
                                        bias_col[:, m:m + 1])
            nc.scalar.add(o_t[:, h:], ps[m][:, h:], bias_col[:, m:m + 1])
            outs.append(o_t)
        for m in range(OT):
            # outputs ride the HWDGE rings, whose inputs are long done
            eng = nc.sync if m % 2 == 0 else nc.scalar
            eng.dma_start(o_d[m * P:(m + 1) * P, :], outs[m][:])

    nc.compile()
    return nc


def _pack(a2d, np_dt):
    """[2048, 512] (k-major rows) -> SBUF layout [128, 16, 512]."""
    return np.ascontiguousarray(
        a2d.reshape(KT, P, -1).transpose(1, 0, 2).astype(np_dt))


def prep_in_maps(input, S, THETA, bias):
    import ml_dtypes

    bf16 = ml_dtypes.bfloat16
    s_np = ml_dtypes.float8_e4m3 if S_MODE == "fp8" else bf16

    xT = input.T  # [2048, 4096] view
    s_host = _pack(S.T, s_np)
    th_host = _pack(THETA.T, bf16)
    b_host = np.ascontiguousarray(bias.reshape(OT, P).T)  # [128, OT]

    return [
        {
            "x": _pack(xT[:, c * B_CORE:(c + 1) * B_CORE], bf16),
            "s": s_host,
            "th": th_host,
            "b": b_host,
        }
        for c in range(N_CORES)
    ]


def gather_out(res):
    out = np.empty((BATCH, OUT_DIM), dtype=np.float32)
    for c in range(N_CORES):
        out[c * B_CORE:(c + 1) * B_CORE, :] = \
            res.results[c]["o"].T.astype(np.float32)
    return out


def _spot_check(out, input, S, THETA, bias):
    """Verify a deterministic sample of output elements on host (a few
    hundred dot products, microseconds) to catch rare transient device
    flakes. Threshold sized for bf16 wire dtypes."""
    rng = np.random.default_rng(1234)
    bs = rng.integers(0, BATCH, size=96)
    os_ = rng.integers(0, OUT_DIM, size=96)
    ref = np.einsum("ij,ij->i", input[bs],
                    S[os_] * THETA[os_]) + bias[os_]
    diff = np.abs(out[bs, os_] - ref)
    scale = np.maximum(1.0, np.abs(ref))
    # per-element: catches garbage; norm: catches broad corruption
    return bool(np.all(diff <= 5e-2 * scale)
                and np.linalg.norm(diff) <= 2e-2 * np.linalg.norm(scale))


def kernel(input, S, THETA, bias):
    from concourse.bass_utils import run_bass_kernel_spmd

    if S_MODE not in _CACHE:
        _CACHE[S_MODE] = _build(S_MODE)
    nc = _CACHE[S_MODE]

    input = np.ascontiguousarray(input, dtype=np.float32)
    S = np.ascontiguousarray(S, dtype=np.float32)
    THETA = np.ascontiguousarray(THETA, dtype=np.float32)
    bias = np.ascontiguousarray(bias, dtype=np.float32)

    in_maps = prep_in_maps(input, S, THETA, bias)
    for _attempt in range(3):
        res = run_bass_kernel_spmd(nc, in_maps, core_ids=list(range(N_CORES)))
        out = gather_out(res)
        if _spot_check(out, input, S, THETA, bias):
            break
    return out
